# revision 4
# baseline (speedup 1.0000x reference)
"""Trainium2 Bass kernel for nn_BanditLayer: out = x @ weight.T + bias.

Full shapes: x [4096, 4096] f32, weight [8192, 4096] f32, bias [8192] f32,
out [4096, 8192] f32.

Sharding: tensor-parallel over output columns. weight/bias are split into 8
slices of 1024 columns; every core holds the full x and computes its own
[4096, 1024] output slice independently (no collectives). The host
pre-transposes x and each weight shard so both matmul operands arrive in
DRAM with the contraction dim (K) outermost — every DMA is then a natural
strided load, no on-chip transposes.

Per-core compute: out_slice = x @ w_slice.T + b_slice as a tiled matmul,
lhsT = xT tile [k128, m128] (stationary), rhs = wT tile [k128, n512]
(moving), accumulating over 32 k-tiles into PSUM [128, 512]. Operands are
bitcast to float32r, which streams at 1 cycle/row (full PE rate, exact
fp32) for moving size >= 256. Bias is added on the vector engine during
PSUM->SBUF eviction from a partition-broadcast bias tile.
"""

import os

import numpy as np

M, K, N = 4096, 4096, 8192
NCORES = 8
NL = N // NCORES  # output cols per core

P = 128  # partitions
NSUB = 512  # moving-operand width (fp32 max)


def build(m=M, k=K, nl=NL):
    from concourse import bacc
    import concourse.mybir as mybir
    from concourse.tile import TileContext

    f32 = mybir.dt.float32
    f32r = mybir.dt.float32r

    mt, kt, nh = m // P, k // P, nl // NSUB
    assert m % P == 0 and k % P == 0 and nl % NSUB == 0

    nc = bacc.Bacc(
        "TRN2", target_bir_lowering=False, debug=False, num_devices=NCORES
    )
    xT = nc.dram_tensor("xT", [k, m], f32r, kind="ExternalInput")
    wT = nc.dram_tensor("wT", [k, nl], f32r, kind="ExternalInput")
    bias = nc.dram_tensor("bias", [nl], f32, kind="ExternalInput")
    out = nc.dram_tensor("out", [m, nl], f32, kind="ExternalOutput")

    with TileContext(nc) as tc:
        with (
            tc.tile_pool(name="wres", bufs=kt) as wpool,
            tc.tile_pool(name="bias", bufs=1) as bpool,
            tc.tile_pool(name="xm", bufs=3) as xpool,
            tc.tile_pool(name="ev", bufs=4) as evpool,
            tc.tile_pool(name="ps", bufs=4, space="PSUM") as pspool,
        ):
            bias_sb = bpool.tile([P, nl], f32)
            nc.sync.dma_start(
                bias_sb[:], bias[:].unsqueeze(0).partition_broadcast(P)
            )

            w_tiles = []
            for ko in range(kt):
                wt = wpool.tile([P, nl], f32r, tag="w")
                nc.sync.dma_start(wt[:], wT[ko * P : (ko + 1) * P, :])
                w_tiles.append(wt)

            for mi in range(mt):
                xm = xpool.tile([P, kt * P], f32r)
                nc.sync.dma_start(
                    xm[:].rearrange("p (ko m) -> p ko m", ko=kt),
                    xT[:, mi * P : (mi + 1) * P].rearrange(
                        "(ko ki) m -> ki ko m", ki=P
                    ),
                )
                for ni in range(nh):
                    ps = pspool.tile([P, NSUB], f32)
                    for ko in range(kt):
                        nc.tensor.matmul(
                            ps[:],
                            xm[:, ko * P : (ko + 1) * P],
                            w_tiles[ko][:, ni * NSUB : (ni + 1) * NSUB],
                            start=(ko == 0),
                            stop=(ko == kt - 1),
                        )
                    ev = evpool.tile([P, NSUB], f32)
                    nc.vector.tensor_add(
                        ev[:], ps[:], bias_sb[:, ni * NSUB : (ni + 1) * NSUB]
                    )
                    nc.sync.dma_start(
                        out[mi * P : (mi + 1) * P, ni * NSUB : (ni + 1) * NSUB],
                        ev[:],
                    )
    nc.compile()
    return nc


def run(x, weight, bias, trace=False):
    """Shard, run on 8 cores, gather. Returns (out, BassKernelResults)."""
    from concourse.bass_utils import run_bass_kernel_spmd

    m, k = x.shape
    n = weight.shape[0]
    nl = n // NCORES
    nc = build(m, k, nl)

    xT = np.ascontiguousarray(x.T)
    in_maps = []
    for c in range(NCORES):
        in_maps.append(
            {
                "xT": xT,
                "wT": np.ascontiguousarray(weight[c * nl : (c + 1) * nl].T),
                "bias": np.ascontiguousarray(bias[c * nl : (c + 1) * nl]),
            }
        )
    res = run_bass_kernel_spmd(
        nc, in_maps, core_ids=list(range(NCORES)), trace=trace
    )
    out = np.concatenate(
        [res.results[i]["out"] for i in range(NCORES)], axis=1
    )
    return out, res


def kernel(x, weight, bias):
    out, _ = run(
        x, weight, bias, trace=bool(os.environ.get("BANDIT_KERNEL_TRACE"))
    )
    return out


# revision 7
# speedup vs baseline: 1.0236x; 1.0236x over previous
"""Trainium2 Bass kernel for nn_BanditLayer: out = x @ weight.T + bias.

Full shapes: x [4096, 4096] f32, weight [8192, 4096] f32, bias [8192] f32,
out [4096, 8192] f32.

Sharding: tensor-parallel over output columns. weight/bias are split into 8
slices of 1024 columns; every core holds the full x and computes its own
[4096, 1024] output slice independently (no collectives).

Layouts: the host pre-transposes/tiles both operands so the contraction dim
(K) lands on SBUF partitions with every DMA a dense, large-descriptor copy:
  x_staged [MT, 128(ki), KT*128(ko,m)]  - 16 KiB contiguous per partition
  w_staged [KG, 128(ki), G*NL(kj,n)]    - 16 KiB contiguous per partition
Matmuls run in float32r (TF32-like, ~1e-4 rel err, 1 PE cycle/row), psum
[128, NL] spans both 512-wide halves; the k-loop interleaves the halves so
consecutive matmuls share the stationary x tile. Bias is added on the
vector engine during PSUM->SBUF eviction; one 512 KiB store per m-tile.

Startup: the first WAVE_G m-tiles run k-major in a staggered wave across
all psum banks so the PE starts as soon as the first w chunk lands instead
of waiting for the full 16 MiB resident weight load. x loads ride the SP
DMA ring (nc.sync), w/bias/out the ACT ring (nc.scalar).
"""

import os

import numpy as np

M, K, N = 4096, 4096, 8192
NCORES = 8
NL = N // NCORES  # output cols per core

P = 128  # partitions
NSUB = 512  # moving-operand width (fp32 max per matmul)
KGRP = 4  # k-tiles per w DMA chunk
WAVE_G = 4  # m-tiles in the startup wave (each uses NL/512 psum banks)
WAVE_S = 6  # stagger (k-steps) between wave groups


def build(m=M, k=K, nl=NL):
    from concourse import bacc
    import concourse.mybir as mybir
    from concourse.tile import TileContext

    f32 = mybir.dt.float32
    f32r = mybir.dt.float32r

    mt, kt = m // P, k // P
    nh = nl // NSUB  # psum halves per m-tile
    kg = min(KGRP, kt)
    assert kt % kg == 0
    ng = kt // kg  # number of w chunks
    wave_g = min(WAVE_G, mt)

    nc = bacc.Bacc(
        "TRN2", target_bir_lowering=False, debug=False, num_devices=NCORES
    )
    xs = nc.dram_tensor("xs", [mt, P, kt * P], f32r, kind="ExternalInput")
    ws = nc.dram_tensor("ws", [ng, P, kg * nl], f32r, kind="ExternalInput")
    bias = nc.dram_tensor("bias", [nl], f32, kind="ExternalInput")
    out = nc.dram_tensor("out", [m, nl], f32, kind="ExternalOutput")

    with TileContext(nc) as tc:
        with (
            tc.tile_pool(name="wres", bufs=ng) as wpool,
            tc.tile_pool(name="bias", bufs=1) as bpool,
            tc.tile_pool(name="xm", bufs=wave_g) as xpool,
            tc.tile_pool(name="ev", bufs=2) as evpool,
            tc.tile_pool(name="ps", bufs=8 // nh, space="PSUM") as pspool,
        ):
            bias_sb = bpool.tile([P, nl], f32)
            nc.scalar.dma_start(
                bias_sb[:], bias[:].unsqueeze(0).partition_broadcast(P)
            )

            w_tiles = []
            for g in range(ng):
                wt = wpool.tile([P, kg * nl], f32r, tag="w", name=f"w{g}")
                nc.scalar.dma_start(wt[:], ws[g])
                w_tiles.append(wt)

            def w_slice(ko, ni):
                # rhs [128, 512]: chunk ko//kg, sub-tile ko%kg, half ni
                return w_tiles[ko // kg][
                    :, (ko % kg) * nl + ni * NSUB : (ko % kg) * nl + (ni + 1) * NSUB
                ]

            def load_x(mi):
                xm = xpool.tile([P, kt * P], f32r, tag="x", name=f"x{mi}")
                nc.sync.dma_start(xm[:], xs[mi])
                return xm

            def mm(ps, xm, mi, ko, ni):
                nc.tensor.matmul(
                    ps[:, ni * NSUB : (ni + 1) * NSUB],
                    xm[:, ko * P : (ko + 1) * P],
                    w_slice(ko, ni),
                    start=(ko == 0),
                    stop=(ko == kt - 1),
                )

            def evict(ps, mi):
                ev = evpool.tile([P, nl], f32, tag="ev", name=f"ev{mi}")
                nc.vector.tensor_add(ev[:], ps[:], bias_sb[:])
                nc.scalar.dma_start(out[mi * P : (mi + 1) * P, :], ev[:])

            # --- startup wave: first wave_g m-tiles, k-major, staggered ---
            wave_x = [load_x(g) for g in range(wave_g)]
            wave_ps = [pspool.tile([P, nl], f32, tag="ps", name=f"wps{g}") for g in range(wave_g)]
            for step in range(kt + (wave_g - 1) * WAVE_S):
                for g in range(wave_g):
                    ko = step - g * WAVE_S
                    if 0 <= ko < kt:
                        for ni in range(nh):
                            mm(wave_ps[g], wave_x[g], g, ko, ni)
            for g in range(wave_g):
                evict(wave_ps[g], g)

            # --- steady state: m-major ---
            for mi in range(wave_g, mt):
                xm = load_x(mi)
                ps = pspool.tile([P, nl], f32, tag="ps", name=f"ps{mi}")
                for ko in range(kt):
                    for ni in range(nh):
                        mm(ps, xm, mi, ko, ni)
                evict(ps, mi)

    nc.compile()
    return nc


def stage_inputs(x, weight, bias_full):
    """Host-side relayout + shard. Returns in_maps for the 8 cores."""
    m, k = x.shape
    n = weight.shape[0]
    nl = n // NCORES
    mt, kt = m // P, k // P
    kg = min(KGRP, kt)
    ng = kt // kg

    # x_staged[mi, ki, ko*128+mm] = x[mi*128+mm, ko*128+ki]
    xs = np.ascontiguousarray(
        x.reshape(mt, P, kt, P).transpose(0, 3, 2, 1).reshape(mt, P, kt * P)
    )
    in_maps = []
    for c in range(NCORES):
        wc = weight[c * nl : (c + 1) * nl]  # [nl, k]
        wT = wc.T  # [k, nl]
        # w_staged[g, ki, j*nl+n] = wT[(g*kg+j)*128+ki, n]
        ws = np.ascontiguousarray(
            wT.reshape(ng, kg, P, nl).transpose(0, 2, 1, 3).reshape(ng, P, kg * nl)
        )
        in_maps.append(
            {
                "xs": xs,
                "ws": ws,
                "bias": np.ascontiguousarray(bias_full[c * nl : (c + 1) * nl]),
            }
        )
    return in_maps


def run(x, weight, bias, trace=False):
    """Shard, run on 8 cores, gather. Returns (out, BassKernelResults)."""
    from concourse.bass_utils import run_bass_kernel_spmd

    m, k = x.shape
    n = weight.shape[0]
    nl = n // NCORES
    nc = build(m, k, nl)
    in_maps = stage_inputs(x, weight, bias)
    res = run_bass_kernel_spmd(
        nc, in_maps, core_ids=list(range(NCORES)), trace=trace
    )
    out = np.concatenate(
        [res.results[i]["out"] for i in range(NCORES)], axis=1
    )
    return out, res


def kernel(x, weight, bias):
    out, _ = run(
        x, weight, bias, trace=bool(os.environ.get("BANDIT_KERNEL_TRACE"))
    )
    return out
